# revision 16
# baseline (speedup 1.0000x reference)
"""Trainium2 Bass kernel for DPMultiheadAttention (L=2048, B=2, E=1024, H=16).

Sharding: batch*head parallel across 8 cores. Core c handles batch c%2 and
heads [4*(c//2), 4*(c//2)+4). Each core computes q/k/v projections for its
256-feature slice, per-head attention, and a partial out-projection; the host
sums the per-batch partials.

v2 design: the Scalar-engine exp wall (128 activations x ~1.15us = ~147us) is
the hard floor on TRN2 (PSUM matmul output must be fp32, so exp instructions
cap at N=1024 within the PSUM budget). The kernel is one flat software-
pipelined loop over 8 blocks x 16 key-chunks; every other PE work item
(q/k/v projections, out-projection) is emitted as a "filler" inside the loop
so the PE never starves the exp stream and the exp stream hides all of it.

  - Scores are row-tiled (tile_position (0,0)/(64,0)): the two heads of a
    pair contract over their 64-dim halves in the two 64-row groups of the
    PE array concurrently - 2x throughput on the D=64 contraction.
  - Context keeps the padded-M scheme: lhsT = [V_h | ones] (M=65), so the
    softmax denominators accumulate for free in PSUM row 64.
  - Normalization: denominator row is DMA-spread to [32,16], reciprocated
    with the fast custom DVE op, DMA-gathered back with a 64-partition
    broadcast, then a single tensor-tensor multiply writes normalized ctx^T.
"""

import numpy as np

import concourse.bass as bass
import concourse.tile as tile
from concourse import mybir
from concourse.bass_utils import run_bass_kernel_spmd

L = 2048
B = 2
E = 1024
H = 16
D = 64
NCORES = 8
HPC = H // NCORES * B  # heads per core = 4
FL = HPC * D  # local feature slice = 256
P = 128

BF16 = mybir.dt.bfloat16
FP32 = mybir.dt.float32

TRACE = False
TRACE_KWARGS = {}
LAST_RESULTS = None


class PatchedTileContext(tile.TileContext):
    """This walrus build caps sync-wait slots per instruction at one; Tile's
    sem assigner freely attaches several. Split extra waits onto same-engine
    nops inserted just before the owning instruction."""

    MAX_WAITS = 1

    def _split_inst_waits(self, inst, out_list):
        si = getattr(inst, "sync_info", None)
        if si is not None and len(si.on_wait) > self.MAX_WAITS:
            waits = list(si.on_wait)
            keep = len(waits) - self.MAX_WAITS
            for i in range(0, keep, self.MAX_WAITS):
                out_list.append(
                    mybir.InstNoOp(
                        name=f"I-ws-{self.nc.next_id()}",
                        engine=inst.engine,
                        bass_nofuse=True,
                        sync_info=mybir.SyncInfo(
                            on_wait=waits[i : i + self.MAX_WAITS], on_update=[]
                        ),
                    )
                )
            inst.sync_info = mybir.SyncInfo(
                on_wait=waits[keep:], on_update=list(si.on_update)
            )
        out_list.append(inst)

    def _lower_ordered_insts(self, ordered):
        for insts in ordered.values():
            new_list = []
            for inst in insts:
                self._split_inst_waits(inst, new_list)
            insts[:] = new_list
        super()._lower_ordered_insts(ordered)

    def _drain_and_barrier(self, tick_clock, wait_clock):
        from bass_rust import SyncInfo
        from concourse.vector_clock import ScopedClock

        drain_inst = self.nc.sync.drain()
        wait_clock.add_sem_waits(
            drain_inst.ins, ScopedClock({None: tick_clock.global_clock})
        )
        si = drain_inst.ins.sync_info
        if si is not None and len(si.on_wait) > self.MAX_WAITS:
            waits = list(si.on_wait)
            drain_inst.ins.sync_info = SyncInfo(
                on_wait=waits[: self.MAX_WAITS], on_update=list(si.on_update)
            )
            for i in range(self.MAX_WAITS, len(waits), self.MAX_WAITS):
                nop = self.nc.sync.nop(nofuse=True)
                nop.ins.sync_info = SyncInfo(
                    on_wait=waits[i : i + self.MAX_WAITS], on_update=[]
                )

        self.nc.all_engine_barrier()
        assert self.sems is not None
        popped = self.nc._tile_sem_poison_stack.pop()
        assert popped is self._sem_poison
        self.nc.clear_and_free_semaphores(list(self.sems.allocated().values()))
        self.nc.all_engine_barrier()


def _ap3(ap, dims):
    return bass.AP(tensor=ap.tensor, offset=ap.offset, ap=dims)


def _bcast_ap(t):
    """DRAM 1-D tensor -> (128, len) partition-broadcast AP for DMA."""
    ap = t[:]
    return bass.AP(tensor=ap.tensor, offset=ap.offset, ap=[[0, P], *ap.ap])


KT = E // P  # 8 contraction tiles for projections
MT = FL // P  # 2 feature tiles (head pairs)
NQ = L // 512  # 4 token chunks of 512
LT = L // P  # 16 token tiles of 128
QB = 4  # q-blocks of 512 per pair
VW = 72  # v_sb inner stride (64 dims + ones col + pad)

EXPF = mybir.ActivationFunctionType.Exp


def build_nc():
    nc = bass.Bass()

    xq = nc.declare_dram_parameter("xq_t", [E, L], BF16, isOutput=False)
    xk = nc.declare_dram_parameter("xk_t", [E, L], BF16, isOutput=False)
    xv = nc.declare_dram_parameter("xv_t", [E, L], BF16, isOutput=False)
    wq = nc.declare_dram_parameter("wq_t", [E, FL], BF16, isOutput=False)
    wk = nc.declare_dram_parameter("wk_t", [E, FL], BF16, isOutput=False)
    wv = nc.declare_dram_parameter("wv_t", [E, FL], BF16, isOutput=False)
    wo = nc.declare_dram_parameter("wo_t", [FL, E], BF16, isOutput=False)
    bq = nc.declare_dram_parameter("bq", [FL], FP32, isOutput=False)
    bk = nc.declare_dram_parameter("bk", [FL], FP32, isOutput=False)
    bv = nc.declare_dram_parameter("bv", [FL], FP32, isOutput=False)
    bo = nc.declare_dram_parameter("bo", [E], FP32, isOutput=False)
    out = nc.declare_dram_parameter("out_p", [L, E], FP32, isOutput=True)

    with PatchedTileContext(nc) as tc:
        with (
            tc.tile_pool(name="singles", bufs=1) as singles,
            tc.tile_pool(name="pt", bufs=4) as pt_pool,
            tc.tile_pool(name="nm", bufs=2) as nm_pool,
            tc.tile_pool(name="rs", bufs=2) as rs_pool,
            tc.tile_pool(name="rb", bufs=2) as rb_pool,
            tc.tile_pool(name="ob", bufs=2) as ob_pool,
            tc.tile_pool(name="sps", bufs=2, space="PSUM") as s_pool,
            tc.tile_pool(name="cps", bufs=2, space="PSUM") as c_pool,
            tc.tile_pool(name="pcs", bufs=2, space="PSUM") as pc_pool,
        ):
            # ---- persistent weights / activations ----
            wq_sb = singles.tile([P, KT, FL], BF16, tag="wq")
            wk_sb = singles.tile([P, KT, FL], BF16, tag="wk")
            wv_sb = singles.tile([P, KT, FL], BF16, tag="wv")
            wo_sb = singles.tile([P, MT, E], BF16, tag="wo")
            bq_sb = singles.tile([P, MT], FP32, tag="bq")
            bk_sb = singles.tile([P, MT], FP32, tag="bk")
            bv_sb = singles.tile([P, FL], FP32, tag="bv")
            bo_sb = singles.tile([P, E], FP32, tag="bo")

            xq_sb = singles.tile([P, KT, L], BF16, tag="xq")
            xk_sb = singles.tile([P, KT, L], BF16, tag="xk")
            xv_sb = singles.tile([P, KT, L], BF16, tag="xv")

            qtp = singles.tile([P, HPC, L], BF16, tag="qtp")
            kt_sb = singles.tile([P, MT, L], BF16, tag="kt")
            v_sb = singles.tile([P, LT, HPC, VW], BF16, tag="v")
            ctx_sb = singles.tile([P, MT, L], BF16, tag="ctx")

            # ones column for the denominator trick
            nc.vector.memset(v_sb[:, :, :, D : D + 1], 1.0)
            # preload the exp table set (~2.7us) while DMAs stream in
            scr = singles.tile([1, 1], FP32, tag="scr")
            nc.scalar.activation(scr[:], v_sb[0:1, 0, 0, D : D + 1], EXPF)

            # ---- DMA issue (one queue; need-order is bandwidth-order) ----
            xq_re = xq.rearrange("(o p) m -> p o m", p=P)
            xk_re = xk.rearrange("(o p) m -> p o m", p=P)
            xv_re = xv.rearrange("(o p) m -> p o m", p=P)

            # loads go on the GpSimd-triggered DMA queue so the Sync queue
            # stays free for the latency-critical normalization DMAs.
            def xdma(sb, re, half):
                # per-k half-L chunks: 2 KiB contiguous lines = full DMA BW
                for k in range(KT):
                    nc.gpsimd.dma_start(
                        sb[:, k, bass.ts(half, 1024)], re[:, k, bass.ts(half, 1024)]
                    )

            gdma = nc.gpsimd.dma_start
            gdma(wq_sb[:], wq.rearrange("(o p) f -> p o f", p=P))
            gdma(bq_sb[:], bq.rearrange("(o p) -> p o", p=P))
            gdma(wk_sb[:], wk.rearrange("(o p) f -> p o f", p=P))
            gdma(bk_sb[:], bk.rearrange("(o p) -> p o", p=P))
            xdma(xq_sb, xq_re, 0)
            xdma(xk_sb, xk_re, 0)
            gdma(wv_sb[:], wv.rearrange("(o p) f -> p o f", p=P))
            gdma(bv_sb[:], _bcast_ap(bv))
            xdma(xv_sb, xv_re, 0)
            xdma(xk_sb, xk_re, 1)
            xdma(xv_sb, xv_re, 1)
            xdma(xq_sb, xq_re, 1)
            gdma(wo_sb[:], wo.rearrange("(o p) f -> p o f", p=P))
            gdma(bo_sb[:], _bcast_ap(bo))

            # ---- work units (PE fillers + their DVE/DMA tails) ----
            def q_unit(mt, nq):
                ps = pc_pool.tile([P, 512], FP32, tag="pc", name=f"psq_{mt}_{nq}")
                for k in range(KT):
                    nc.tensor.matmul(
                        ps[:],
                        wq_sb[:, k, bass.ts(mt, P)],
                        xq_sb[:, k, bass.ts(nq, 512)],
                        start=(k == 0),
                        stop=(k == KT - 1),
                    )
                nc.vector.tensor_scalar_add(
                    qtp[0:D, 2 * mt, bass.ts(nq, 512)],
                    ps[0:D],
                    bq_sb[0:D, mt : mt + 1],
                )
                nc.vector.tensor_scalar_add(
                    qtp[D:P, 2 * mt + 1, bass.ts(nq, 512)],
                    ps[D:P],
                    bq_sb[D:P, mt : mt + 1],
                )

            def k_unit(mt, nq):
                ps = pc_pool.tile([P, 512], FP32, tag="pc", name=f"psk_{mt}_{nq}")
                for k in range(KT):
                    nc.tensor.matmul(
                        ps[:],
                        wk_sb[:, k, bass.ts(mt, P)],
                        xk_sb[:, k, bass.ts(nq, 512)],
                        start=(k == 0),
                        stop=(k == KT - 1),
                    )
                nc.vector.tensor_scalar_add(
                    kt_sb[:, mt, bass.ts(nq, 512)], ps[:], bk_sb[:, mt : mt + 1]
                )

            def v_unit(pair, lt):
                ps = pc_pool.tile([P, 512], FP32, tag="pc", name=f"psv_{pair}_{lt}")
                for k in range(KT):
                    nc.tensor.matmul(
                        ps[:, 0:P],
                        xv_sb[:, k, bass.ts(lt, P)],
                        wv_sb[:, k, bass.ds(pair * P, P)],
                        start=(k == 0),
                        stop=(k == KT - 1),
                    )
                nc.vector.tensor_add(
                    v_sb[:, lt, 2 * pair : 2 * pair + 2, 0:D],
                    ps[:, 0:P].rearrange("p (h d) -> p h d", d=D),
                    bv_sb[:, bass.ds(pair * P, P)].rearrange("p (h d) -> p h d", d=D),
                )

            def c_unit(lt):
                osb = ob_pool.tile([P, E], FP32, tag="osb", name=f"osb_{lt}")
                for nn in range(2):
                    ps = pc_pool.tile([P, 512], FP32, tag="pc", name=f"psc_{lt}_{nn}")
                    for kt in range(MT):
                        nc.tensor.matmul(
                            ps[:],
                            ctx_sb[:, kt, bass.ts(lt, P)],
                            wo_sb[:, kt, bass.ds(nn * 512, 512)],
                            start=(kt == 0),
                            stop=(kt == MT - 1),
                        )
                    nc.vector.tensor_add(
                        osb[:, bass.ds(nn * 512, 512)],
                        ps[:],
                        bo_sb[:, bass.ds(nn * 512, 512)],
                    )
                nc.gpsimd.dma_start(out[bass.ts(lt, P), :], osb[:])

            # filler schedule: iteration index -> list of thunks
            fillers = {t: [] for t in range(129)}

            def sched(t, fn, *args):
                fillers[t].append((fn, args))

            # b0 (p0,qb0): K(mt0) nq1-3 (deadline scores j=4nq), V(p0)
            # lt-aligned (deadline ctx j=lt).  This block is structurally
            # PE-overloaded; everything else is deferred.
            sched(1, k_unit, 0, 1)
            sched(5, k_unit, 0, 2)
            sched(9, k_unit, 0, 3)
            for lt in range(LT):
                sched(lt, v_unit, 0, lt)
            sched(13, q_unit, 0, 1)
            # b1 (p0,qb1)
            sched(18, q_unit, 0, 2)
            sched(22, q_unit, 1, 0)
            sched(26, k_unit, 1, 0)
            # b2 (p0,qb2): V(p1) starts (deadline ctx(b4) at t=64+lt)
            sched(33, q_unit, 0, 3)
            sched(37, k_unit, 1, 1)
            sched(41, k_unit, 1, 2)
            for i in range(5):
                sched(34 + 3 * i, v_unit, 1, i)
            # b3 (p0,qb3)
            sched(49, k_unit, 1, 3)
            sched(53, q_unit, 1, 1)
            for i in range(5, LT):
                sched(44 + 2 * i, v_unit, 1, i)
            # b4 (p1,qb0): light
            sched(70, q_unit, 1, 2)
            # b5..b7 + tail: out-projection; c_unit(qb) needs norm(p1,qb)
            # emitted at iter (4+qb)*16+16, so start at 81 + 16*qb.
            sched(75, q_unit, 1, 3)
            for qb in range(QB):
                for i in range(4):
                    lt = qb * 4 + i
                    sched(min(83 + qb * 16 + i * 3, 128), c_unit, lt)

            def run_fillers(t):
                for fn, args in fillers[t]:
                    fn(*args)

            # ---- pre-phase: warm the PE while xq/xk stream in, then the
            # first q/k units so scores can start ----
            warm = pc_pool.tile([P, 512], FP32, tag="pc", name="warm")
            for i in range(14):
                nc.tensor.matmul(
                    warm[:], wq_sb[:, 0, 0:P], wq_sb[:, 0:4, :].rearrange(
                        "p a b -> p (a b)"
                    )[:, 0:512], start=True, stop=True,
                )
            q_unit(0, 0)
            k_unit(0, 0)

            # ---- flat pipelined loop ----
            blocks = [(pair, qb) for pair in range(MT) for qb in range(QB)]
            prev = None  # (pair, qb, j, pts, cps)
            cps = None

            def emit_ctx(state):
                pair_, qb_, j_, pts_, cps_ = state
                for hh in range(2):
                    h = 2 * pair_ + hh
                    nc.tensor.matmul(
                        cps_[hh][0 : D + 1, :],
                        v_sb[:, j_, h, 0 : D + 1],
                        pts_[:, bass.ts(hh, 512)],
                        start=(j_ == 0),
                        stop=(j_ == LT - 1),
                    )

            def emit_norm(state):
                pair_, qb_, j_, pts_, cps_ = state
                craw = nm_pool.tile(
                    [D + 1, 1024], FP32, tag="craw", name=f"craw_{pair_}_{qb_}"
                )
                for hh in range(2):
                    nc.vector.tensor_copy(
                        craw[:, bass.ts(hh, 512)], cps_[hh][0 : D + 1, :]
                    )
                # spread the 1024 denominators (both heads) over 32 partitions
                rsp = rs_pool.tile([32, 32], FP32, tag="rsp")
                dr = craw[D : D + 1, :]
                nc.sync.dma_start(
                    out=rsp[:],
                    in_=_ap3(dr, [dr.ap[0], [32, 32], [1, 32]]),
                )
                rrec = rs_pool.tile([32, 32], FP32, tag="rrec")
                nc.vector.reciprocal(rrec[:], rsp[:])
                # gather back to one partition, then broadcast down 64
                rrow = rs_pool.tile([1, 1024], FP32, tag="rrow")
                nc.sync.dma_start(
                    out=_ap3(rrow[:], [rrow[:].ap[0], [32, 32], [1, 32]]),
                    in_=rrec[:],
                )
                rb = rb_pool.tile([D, 1024], FP32, tag="rb")
                rap = rrow[0:1, :]
                nc.sync.dma_start(
                    out=_ap3(rb[:], [rb[:].ap[0], [1, 1], rb[:].ap[1]]),
                    in_=_ap3(rap, [[1, 1], [0, D], rap.ap[-1]]),
                )
                for hh in range(2):
                    nc.vector.tensor_mul(
                        ctx_sb[D * hh : D * hh + D, pair_, bass.ds(qb_ * 512, 512)],
                        craw[0:D, bass.ts(hh, 512)],
                        rb[:, bass.ts(hh, 512)],
                    )

            for t in range(128):
                bi, j = t // LT, t % LT
                pair, qb = blocks[bi]
                if j == 0:
                    cps = [
                        c_pool.tile([P, 512], FP32, tag="c", name=f"cps_{bi}_{hh}")
                        for hh in range(2)
                    ]
                # scores: both heads concurrently via row tiling
                sps = s_pool.tile([P, 1024], FP32, tag="s", name=f"sps_{t}")
                for hh in range(2):
                    nc.tensor.matmul(
                        sps[:, bass.ts(hh, 512)],
                        kt_sb[D * hh : D * hh + D, pair, bass.ts(j, P)],
                        qtp[D * hh : D * hh + D, 2 * pair + hh, bass.ds(qb * 512, 512)],
                        start=True,
                        stop=True,
                        tile_position=(D * hh, 0),
                    )
                pts = pt_pool.tile([P, 1024], BF16, tag="pt", name=f"pt_{t}")
                nc.scalar.activation(pts[:], sps[:], EXPF)
                run_fillers(t)
                if prev is not None:
                    emit_ctx(prev)
                    if prev[2] == LT - 1:
                        emit_norm(prev)
                prev = (pair, qb, j, pts, cps)

            # ---- tail ----
            emit_ctx(prev)
            emit_norm(prev)
            # keep the PE clock warm through the norm-chain wait so the last
            # out-projection units run at 2.4 GHz
            warm2 = pc_pool.tile([P, 512], FP32, tag="pc", name="warm2")
            for i in range(16):
                nc.tensor.matmul(
                    warm2[:],
                    kt_sb[:, 0, 0:P],
                    qtp[:, 0, 0:512],
                    start=True,
                    stop=True,
                )
            run_fillers(128)

    return nc


_NC = None


def _get_nc():
    global _NC
    if _NC is None:
        _NC = build_nc()
    return _NC


def kernel(query, key, value, w_in, b_in, w_out, b_out):
    import ml_dtypes

    bf16 = ml_dtypes.bfloat16
    query = np.asarray(query, dtype=np.float32)
    key = np.asarray(key, dtype=np.float32)
    value = np.asarray(value, dtype=np.float32)
    w_in = np.asarray(w_in, dtype=np.float32)
    b_in = np.asarray(b_in, dtype=np.float32)
    w_out = np.asarray(w_out, dtype=np.float32)
    b_out = np.asarray(b_out, dtype=np.float32)

    scale = float(D) ** -0.5
    in_maps = []
    for c in range(NCORES):
        b = c % 2
        g = c // 2
        sl = slice(FL * g, FL * (g + 1))
        wq = w_in[0 * E : 1 * E][sl] * scale  # (256, 1024)
        wk = w_in[1 * E : 2 * E][sl]
        wv = w_in[2 * E : 3 * E][sl]
        in_maps.append(
            {
                "xq_t": np.ascontiguousarray(query[:, b, :].T).astype(bf16),
                "xk_t": np.ascontiguousarray(key[:, b, :].T).astype(bf16),
                "xv_t": np.ascontiguousarray(value[:, b, :].T).astype(bf16),
                "wq_t": np.ascontiguousarray(wq.T).astype(bf16),
                "wk_t": np.ascontiguousarray(wk.T).astype(bf16),
                "wv_t": np.ascontiguousarray(wv.T).astype(bf16),
                "wo_t": np.ascontiguousarray(w_out[:, sl].T).astype(bf16),
                "bq": np.ascontiguousarray(b_in[0 * E : 1 * E][sl] * scale),
                "bk": np.ascontiguousarray(b_in[1 * E : 2 * E][sl]),
                "bv": np.ascontiguousarray(b_in[2 * E : 3 * E][sl]),
                "bo": b_out if c < 2 else np.zeros_like(b_out),
            }
        )

    nc = _get_nc()
    res = run_bass_kernel_spmd(
        nc, in_maps, list(range(NCORES)), trace=TRACE, **TRACE_KWARGS
    )
    global LAST_RESULTS
    LAST_RESULTS = res

    out = np.zeros((L, B, E), dtype=np.float32)
    for c in range(NCORES):
        out[:, c % 2, :] += res.results[c]["out_p"]
    return out


# revision 19
# speedup vs baseline: 1.0194x; 1.0194x over previous
"""Trainium2 Bass kernel for DPMultiheadAttention (L=2048, B=2, E=1024, H=16).

Sharding: batch*head parallel across 8 cores. Core c handles batch c%2 and
heads [4*(c//2), 4*(c//2)+4). Each core computes q/k/v projections for its
256-feature slice, per-head attention, and a partial out-projection; the host
sums the per-batch partials.

v2 design: the Scalar-engine exp wall (128 activations x ~1.15us = ~147us) is
the hard floor on TRN2 (PSUM matmul output must be fp32, so exp instructions
cap at N=1024 within the PSUM budget). The kernel is one flat software-
pipelined loop over 8 blocks x 16 key-chunks; every other PE work item
(q/k/v projections, out-projection) is emitted as a "filler" inside the loop
so the PE never starves the exp stream and the exp stream hides all of it.

  - Scores are row-tiled (tile_position (0,0)/(64,0)): the two heads of a
    pair contract over their 64-dim halves in the two 64-row groups of the
    PE array concurrently - 2x throughput on the D=64 contraction.
  - Context keeps the padded-M scheme: lhsT = [V_h | ones] (M=65), so the
    softmax denominators accumulate for free in PSUM row 64.
  - Normalization: denominator row is DMA-spread to [32,16], reciprocated
    with the fast custom DVE op, DMA-gathered back with a 64-partition
    broadcast, then a single tensor-tensor multiply writes normalized ctx^T.
"""

import numpy as np

import concourse.bass as bass
import concourse.tile as tile
from concourse import mybir
from concourse.bass_utils import run_bass_kernel_spmd

L = 2048
B = 2
E = 1024
H = 16
D = 64
NCORES = 8
HPC = H // NCORES * B  # heads per core = 4
FL = HPC * D  # local feature slice = 256
P = 128

BF16 = mybir.dt.bfloat16
FP32 = mybir.dt.float32

TRACE = False
TRACE_KWARGS = {}
LAST_RESULTS = None


class PatchedTileContext(tile.TileContext):
    """This walrus build caps sync-wait slots per instruction at one; Tile's
    sem assigner freely attaches several. Split extra waits onto same-engine
    nops inserted just before the owning instruction."""

    MAX_WAITS = 1

    def _split_inst_waits(self, inst, out_list):
        si = getattr(inst, "sync_info", None)
        if si is not None and len(si.on_wait) > self.MAX_WAITS:
            waits = list(si.on_wait)
            keep = len(waits) - self.MAX_WAITS
            for i in range(0, keep, self.MAX_WAITS):
                out_list.append(
                    mybir.InstNoOp(
                        name=f"I-ws-{self.nc.next_id()}",
                        engine=inst.engine,
                        bass_nofuse=True,
                        sync_info=mybir.SyncInfo(
                            on_wait=waits[i : i + self.MAX_WAITS], on_update=[]
                        ),
                    )
                )
            inst.sync_info = mybir.SyncInfo(
                on_wait=waits[keep:], on_update=list(si.on_update)
            )
        out_list.append(inst)

    def _lower_ordered_insts(self, ordered):
        for insts in ordered.values():
            new_list = []
            for inst in insts:
                self._split_inst_waits(inst, new_list)
            insts[:] = new_list
        super()._lower_ordered_insts(ordered)

    def _drain_and_barrier(self, tick_clock, wait_clock):
        from bass_rust import SyncInfo
        from concourse.vector_clock import ScopedClock

        drain_inst = self.nc.sync.drain()
        wait_clock.add_sem_waits(
            drain_inst.ins, ScopedClock({None: tick_clock.global_clock})
        )
        si = drain_inst.ins.sync_info
        if si is not None and len(si.on_wait) > self.MAX_WAITS:
            waits = list(si.on_wait)
            drain_inst.ins.sync_info = SyncInfo(
                on_wait=waits[: self.MAX_WAITS], on_update=list(si.on_update)
            )
            for i in range(self.MAX_WAITS, len(waits), self.MAX_WAITS):
                nop = self.nc.sync.nop(nofuse=True)
                nop.ins.sync_info = SyncInfo(
                    on_wait=waits[i : i + self.MAX_WAITS], on_update=[]
                )

        self.nc.all_engine_barrier()
        assert self.sems is not None
        popped = self.nc._tile_sem_poison_stack.pop()
        assert popped is self._sem_poison
        self.nc.clear_and_free_semaphores(list(self.sems.allocated().values()))
        self.nc.all_engine_barrier()


def _ap3(ap, dims):
    return bass.AP(tensor=ap.tensor, offset=ap.offset, ap=dims)


def _bcast_ap(t):
    """DRAM 1-D tensor -> (128, len) partition-broadcast AP for DMA."""
    ap = t[:]
    return bass.AP(tensor=ap.tensor, offset=ap.offset, ap=[[0, P], *ap.ap])


KT = E // P  # 8 contraction tiles for projections
MT = FL // P  # 2 feature tiles (head pairs)
NQ = L // 512  # 4 token chunks of 512
LT = L // P  # 16 token tiles of 128
QB = 4  # q-blocks of 512 per pair
VW = 72  # v_sb inner stride (64 dims + ones col + pad)

EXPF = mybir.ActivationFunctionType.Exp


def build_nc():
    nc = bass.Bass()

    xq = nc.declare_dram_parameter("xq_t", [E, L], BF16, isOutput=False)
    xk = nc.declare_dram_parameter("xk_t", [E, L], BF16, isOutput=False)
    xv = nc.declare_dram_parameter("xv_t", [E, L], BF16, isOutput=False)
    wq = nc.declare_dram_parameter("wq_t", [E, FL], BF16, isOutput=False)
    wk = nc.declare_dram_parameter("wk_t", [E, FL], BF16, isOutput=False)
    wv = nc.declare_dram_parameter("wv_t", [E, FL], BF16, isOutput=False)
    wo = nc.declare_dram_parameter("wo_t", [FL, E], BF16, isOutput=False)
    bq = nc.declare_dram_parameter("bq", [FL], FP32, isOutput=False)
    bk = nc.declare_dram_parameter("bk", [FL], FP32, isOutput=False)
    bv = nc.declare_dram_parameter("bv", [FL], FP32, isOutput=False)
    bo = nc.declare_dram_parameter("bo", [E], FP32, isOutput=False)
    out = nc.declare_dram_parameter("out_p", [L, E], FP32, isOutput=True)

    with PatchedTileContext(nc) as tc:
        with (
            tc.tile_pool(name="singles", bufs=1) as singles,
            tc.tile_pool(name="pt", bufs=6) as pt_pool,
            tc.tile_pool(name="nm", bufs=2) as nm_pool,
            tc.tile_pool(name="rs", bufs=2) as rs_pool,
            tc.tile_pool(name="rb", bufs=2) as rb_pool,
            tc.tile_pool(name="ob", bufs=2) as ob_pool,
            tc.tile_pool(name="sps", bufs=2, space="PSUM") as s_pool,
            tc.tile_pool(name="cps", bufs=2, space="PSUM") as c_pool,
            tc.tile_pool(name="pcs", bufs=2, space="PSUM") as pc_pool,
        ):
            # ---- persistent weights / activations ----
            wq_sb = singles.tile([P, KT, FL], BF16, tag="wq")
            wk_sb = singles.tile([P, KT, FL], BF16, tag="wk")
            wv_sb = singles.tile([P, KT, FL], BF16, tag="wv")
            wo_sb = singles.tile([P, MT, E], BF16, tag="wo")
            bq_sb = singles.tile([P, MT], FP32, tag="bq")
            bk_sb = singles.tile([P, MT], FP32, tag="bk")
            bv_sb = singles.tile([P, FL], FP32, tag="bv")
            bo_sb = singles.tile([P, E], FP32, tag="bo")

            xq_sb = singles.tile([P, KT, L], BF16, tag="xq")
            xk_sb = singles.tile([P, KT, L], BF16, tag="xk")
            xv_sb = singles.tile([P, KT, L], BF16, tag="xv")

            qtp = singles.tile([P, HPC, L], BF16, tag="qtp")
            kt_sb = singles.tile([P, MT, L], BF16, tag="kt")
            v_sb = singles.tile([P, LT, HPC, VW], BF16, tag="v")
            ctx_sb = singles.tile([P, MT, L], BF16, tag="ctx")

            # ones column for the denominator trick
            nc.vector.memset(v_sb[:, :, :, D : D + 1], 1.0)
            # preload the exp table set (~2.7us) while DMAs stream in
            scr = singles.tile([1, 1], FP32, tag="scr")
            nc.scalar.activation(scr[:], v_sb[0:1, 0, 0, D : D + 1], EXPF)

            # ---- DMA issue (one queue; need-order is bandwidth-order) ----
            xq_re = xq.rearrange("(o p) m -> p o m", p=P)
            xk_re = xk.rearrange("(o p) m -> p o m", p=P)
            xv_re = xv.rearrange("(o p) m -> p o m", p=P)

            # Loads use the Sync HWDGE ring (fast RTL dispatch), one 1 MiB
            # chunk per (tensor, 512-token block), ordered by first use.
            # The norm DMAs share this ring but only the p1-block ones are
            # latency-critical, and those run after the loads finish.
            def xdma(sb, re, nq):
                nc.sync.dma_start(
                    sb[:, :, bass.ts(nq, 512)], re[:, :, bass.ts(nq, 512)]
                )

            nc.sync.dma_start(wq_sb[:], wq.rearrange("(o p) f -> p o f", p=P))
            nc.sync.dma_start(bq_sb[:], bq.rearrange("(o p) -> p o", p=P))
            nc.sync.dma_start(wk_sb[:], wk.rearrange("(o p) f -> p o f", p=P))
            nc.sync.dma_start(bk_sb[:], bk.rearrange("(o p) -> p o", p=P))
            xdma(xq_sb, xq_re, 0)
            xdma(xk_sb, xk_re, 0)
            nc.sync.dma_start(wv_sb[:], wv.rearrange("(o p) f -> p o f", p=P))
            nc.sync.dma_start(bv_sb[:], _bcast_ap(bv))
            xdma(xv_sb, xv_re, 0)
            xdma(xk_sb, xk_re, 1)
            xdma(xv_sb, xv_re, 1)
            xdma(xk_sb, xk_re, 2)
            xdma(xv_sb, xv_re, 2)
            xdma(xk_sb, xk_re, 3)
            xdma(xv_sb, xv_re, 3)
            xdma(xq_sb, xq_re, 1)
            nc.sync.dma_start(wo_sb[:], wo.rearrange("(o p) f -> p o f", p=P))
            xdma(xq_sb, xq_re, 2)
            xdma(xq_sb, xq_re, 3)
            nc.sync.dma_start(bo_sb[:], _bcast_ap(bo))

            # ---- work units (PE fillers + their DVE/DMA tails) ----
            def q_unit(mt, nq):
                ps = pc_pool.tile([P, 512], FP32, tag="pc", name=f"psq_{mt}_{nq}")
                for k in range(KT):
                    nc.tensor.matmul(
                        ps[:],
                        wq_sb[:, k, bass.ts(mt, P)],
                        xq_sb[:, k, bass.ts(nq, 512)],
                        start=(k == 0),
                        stop=(k == KT - 1),
                    )
                nc.vector.tensor_scalar_add(
                    qtp[0:D, 2 * mt, bass.ts(nq, 512)],
                    ps[0:D],
                    bq_sb[0:D, mt : mt + 1],
                )
                nc.vector.tensor_scalar_add(
                    qtp[D:P, 2 * mt + 1, bass.ts(nq, 512)],
                    ps[D:P],
                    bq_sb[D:P, mt : mt + 1],
                )

            def k_unit(mt, nq):
                ps = pc_pool.tile([P, 512], FP32, tag="pc", name=f"psk_{mt}_{nq}")
                for k in range(KT):
                    nc.tensor.matmul(
                        ps[:],
                        wk_sb[:, k, bass.ts(mt, P)],
                        xk_sb[:, k, bass.ts(nq, 512)],
                        start=(k == 0),
                        stop=(k == KT - 1),
                    )
                nc.vector.tensor_scalar_add(
                    kt_sb[:, mt, bass.ts(nq, 512)], ps[:], bk_sb[:, mt : mt + 1]
                )

            def v_unit(pair, lt):
                ps = pc_pool.tile([P, 512], FP32, tag="pc", name=f"psv_{pair}_{lt}")
                for k in range(KT):
                    nc.tensor.matmul(
                        ps[:, 0:P],
                        xv_sb[:, k, bass.ts(lt, P)],
                        wv_sb[:, k, bass.ds(pair * P, P)],
                        start=(k == 0),
                        stop=(k == KT - 1),
                    )
                nc.vector.tensor_add(
                    v_sb[:, lt, 2 * pair : 2 * pair + 2, 0:D],
                    ps[:, 0:P].rearrange("p (h d) -> p h d", d=D),
                    bv_sb[:, bass.ds(pair * P, P)].rearrange("p (h d) -> p h d", d=D),
                )

            def c_unit(lt):
                osb = ob_pool.tile([P, E], FP32, tag="osb", name=f"osb_{lt}")
                for nn in range(2):
                    ps = pc_pool.tile([P, 512], FP32, tag="pc", name=f"psc_{lt}_{nn}")
                    for kt in range(MT):
                        nc.tensor.matmul(
                            ps[:],
                            ctx_sb[:, kt, bass.ts(lt, P)],
                            wo_sb[:, kt, bass.ds(nn * 512, 512)],
                            start=(kt == 0),
                            stop=(kt == MT - 1),
                        )
                    nc.vector.tensor_add(
                        osb[:, bass.ds(nn * 512, 512)],
                        ps[:],
                        bo_sb[:, bass.ds(nn * 512, 512)],
                    )
                nc.gpsimd.dma_start(out[bass.ts(lt, P), :], osb[:])

            # filler schedule: iteration index -> list of thunks
            fillers = {t: [] for t in range(129)}

            def sched(t, fn, *args):
                fillers[t].append((fn, args))

            # b0 (p0,qb0): K(mt0) nq1-3 (deadline scores j=4nq), V(p0)
            # lt-aligned (deadline ctx j=lt).  This block is structurally
            # PE-overloaded; everything else is deferred.
            sched(1, k_unit, 0, 1)
            sched(5, k_unit, 0, 2)
            sched(9, k_unit, 0, 3)
            for lt in range(LT):
                sched(lt, v_unit, 0, lt)
            sched(13, q_unit, 0, 1)
            # b1 (p0,qb1)
            sched(18, q_unit, 0, 2)
            sched(22, q_unit, 1, 0)
            sched(26, k_unit, 1, 0)
            # b2 (p0,qb2): V(p1) starts (deadline ctx(b4) at t=64+lt)
            sched(33, q_unit, 0, 3)
            sched(37, k_unit, 1, 1)
            sched(41, k_unit, 1, 2)
            for i in range(5):
                sched(34 + 3 * i, v_unit, 1, i)
            # b3 (p0,qb3)
            sched(49, k_unit, 1, 3)
            sched(53, q_unit, 1, 1)
            for i in range(5, LT):
                sched(44 + 2 * i, v_unit, 1, i)
            # b4 (p1,qb0): light
            sched(70, q_unit, 1, 2)
            # b5..b7 + tail: out-projection; c_unit(qb) needs norm(p1,qb)
            # emitted at iter (4+qb)*16+16, so start at 81 + 16*qb.
            sched(75, q_unit, 1, 3)
            for qb in range(QB):
                for i in range(4):
                    lt = qb * 4 + i
                    sched(min(88 + qb * 16 + i * 2, 128), c_unit, lt)

            def run_fillers(t):
                for fn, args in fillers[t]:
                    fn(*args)

            # ---- pre-phase: warm the PE while xq/xk stream in, then the
            # first q/k units so scores can start ----
            warm = pc_pool.tile([P, 512], FP32, tag="pc", name="warm")
            for i in range(14):
                nc.tensor.matmul(
                    warm[:], wq_sb[:, 0, 0:P], wq_sb[:, 0:4, :].rearrange(
                        "p a b -> p (a b)"
                    )[:, 0:512], start=True, stop=True,
                )
            q_unit(0, 0)
            k_unit(0, 0)

            # ---- flat pipelined loop ----
            blocks = [(pair, qb) for pair in range(MT) for qb in range(QB)]
            prev = None  # (pair, qb, j, pts, cps)
            cps = None

            def emit_ctx(state):
                pair_, qb_, j_, pts_, cps_ = state
                for hh in range(2):
                    h = 2 * pair_ + hh
                    nc.tensor.matmul(
                        cps_[hh][0 : D + 1, :],
                        v_sb[:, j_, h, 0 : D + 1],
                        pts_[:, bass.ts(hh, 512)],
                        start=(j_ == 0),
                        stop=(j_ == LT - 1),
                    )

            def emit_norm(state):
                pair_, qb_, j_, pts_, cps_ = state
                craw = nm_pool.tile(
                    [D + 1, 1024], FP32, tag="craw", name=f"craw_{pair_}_{qb_}"
                )
                for hh in range(2):
                    nc.vector.tensor_copy(
                        craw[:, bass.ts(hh, 512)], cps_[hh][0 : D + 1, :]
                    )
                # spread the 1024 denominators (both heads) over 32 partitions
                rsp = rs_pool.tile([32, 32], FP32, tag="rsp")
                dr = craw[D : D + 1, :]
                nc.sync.dma_start(
                    out=rsp[:],
                    in_=_ap3(dr, [dr.ap[0], [32, 32], [1, 32]]),
                )
                rrec = rs_pool.tile([32, 32], FP32, tag="rrec")
                nc.vector.reciprocal(rrec[:], rsp[:])
                # gather back to one partition, then broadcast down 64
                rrow = rs_pool.tile([1, 1024], FP32, tag="rrow")
                nc.sync.dma_start(
                    out=_ap3(rrow[:], [rrow[:].ap[0], [32, 32], [1, 32]]),
                    in_=rrec[:],
                )
                rb = rb_pool.tile([D, 1024], FP32, tag="rb")
                rap = rrow[0:1, :]
                nc.sync.dma_start(
                    out=_ap3(rb[:], [rb[:].ap[0], [1, 1], rb[:].ap[1]]),
                    in_=_ap3(rap, [[1, 1], [0, D], rap.ap[-1]]),
                )
                for hh in range(2):
                    nc.vector.tensor_mul(
                        ctx_sb[D * hh : D * hh + D, pair_, bass.ds(qb_ * 512, 512)],
                        craw[0:D, bass.ts(hh, 512)],
                        rb[:, bass.ts(hh, 512)],
                    )

            for t in range(128):
                bi, j = t // LT, t % LT
                pair, qb = blocks[bi]
                if j == 0:
                    cps = [
                        c_pool.tile([P, 512], FP32, tag="c", name=f"cps_{bi}_{hh}")
                        for hh in range(2)
                    ]
                # scores: both heads concurrently via row tiling
                sps = s_pool.tile([P, 1024], FP32, tag="s", name=f"sps_{t}")
                for hh in range(2):
                    nc.tensor.matmul(
                        sps[:, bass.ts(hh, 512)],
                        kt_sb[D * hh : D * hh + D, pair, bass.ts(j, P)],
                        qtp[D * hh : D * hh + D, 2 * pair + hh, bass.ds(qb * 512, 512)],
                        start=True,
                        stop=True,
                        tile_position=(D * hh, 0),
                    )
                pts = pt_pool.tile([P, 1024], BF16, tag="pt", name=f"pt_{t}")
                nc.scalar.activation(pts[:], sps[:], EXPF)
                run_fillers(t)
                if prev is not None:
                    emit_ctx(prev)
                    if prev[2] == LT - 1:
                        emit_norm(prev)
                prev = (pair, qb, j, pts, cps)

            # ---- tail ----
            emit_ctx(prev)
            emit_norm(prev)
            # keep the PE clock warm through the norm-chain wait so the last
            # out-projection units run at 2.4 GHz
            warm2 = pc_pool.tile([P, 512], FP32, tag="pc", name="warm2")
            for i in range(16):
                nc.tensor.matmul(
                    warm2[:],
                    kt_sb[:, 0, 0:P],
                    qtp[:, 0, 0:512],
                    start=True,
                    stop=True,
                )
            run_fillers(128)

    return nc


_NC = None


def _get_nc():
    global _NC
    if _NC is None:
        _NC = build_nc()
    return _NC


def kernel(query, key, value, w_in, b_in, w_out, b_out):
    import ml_dtypes

    bf16 = ml_dtypes.bfloat16
    query = np.asarray(query, dtype=np.float32)
    key = np.asarray(key, dtype=np.float32)
    value = np.asarray(value, dtype=np.float32)
    w_in = np.asarray(w_in, dtype=np.float32)
    b_in = np.asarray(b_in, dtype=np.float32)
    w_out = np.asarray(w_out, dtype=np.float32)
    b_out = np.asarray(b_out, dtype=np.float32)

    scale = float(D) ** -0.5
    in_maps = []
    for c in range(NCORES):
        b = c % 2
        g = c // 2
        sl = slice(FL * g, FL * (g + 1))
        wq = w_in[0 * E : 1 * E][sl] * scale  # (256, 1024)
        wk = w_in[1 * E : 2 * E][sl]
        wv = w_in[2 * E : 3 * E][sl]
        in_maps.append(
            {
                "xq_t": np.ascontiguousarray(query[:, b, :].T).astype(bf16),
                "xk_t": np.ascontiguousarray(key[:, b, :].T).astype(bf16),
                "xv_t": np.ascontiguousarray(value[:, b, :].T).astype(bf16),
                "wq_t": np.ascontiguousarray(wq.T).astype(bf16),
                "wk_t": np.ascontiguousarray(wk.T).astype(bf16),
                "wv_t": np.ascontiguousarray(wv.T).astype(bf16),
                "wo_t": np.ascontiguousarray(w_out[:, sl].T).astype(bf16),
                "bq": np.ascontiguousarray(b_in[0 * E : 1 * E][sl] * scale),
                "bk": np.ascontiguousarray(b_in[1 * E : 2 * E][sl]),
                "bv": np.ascontiguousarray(b_in[2 * E : 3 * E][sl]),
                "bo": b_out if c < 2 else np.zeros_like(b_out),
            }
        )

    nc = _get_nc()
    res = run_bass_kernel_spmd(
        nc, in_maps, list(range(NCORES)), trace=TRACE, **TRACE_KWARGS
    )
    global LAST_RESULTS
    LAST_RESULTS = res

    out = np.zeros((L, B, E), dtype=np.float32)
    for c in range(NCORES):
        out[:, c % 2, :] += res.results[c]["out_p"]
    return out
